# revision 1
# baseline (speedup 1.0000x reference)
"""Trainium2 Bass kernel for nn_Conv_MS_MSA (spectral multi-head self-attention).

Reference computation (per batch):
  qkv = dw3x3_depthwise(conv1x1(x))          # 256 -> 768 ch, then per-ch 3x3
  q, k, v = split(qkv); v_out = v
  per head (8 heads x 32 d): L2-normalize q,k rows over the 65536 pixels,
  attn = softmax(k_norm @ q_norm^T * rescale), out = attn @ v
  out_c = conv3x3_dense(out, w_proj)         # 256 -> 256 ch

Sharding: spatial bands. Core i owns image rows [32i, 32i+32) of BOTH batches,
with halo rows for the two 3x3 convs. The only global coupling is the per-head
32x32 Gram matrices and q/k row norms -- tiny sums over pixels -- reduced with
one ~70KB on-device AllReduce mid-kernel. Everything else is band-local.

Layouts on device: channels on SBUF partitions, pixels on the free dim.
Matmuls run in float32r (full PE rate at free-dim>=256); the q/k path runs in
bf16 (the normalized Gram tolerates it; v and the proj path stay fp32r). The
depthwise conv is split tap-wise across PE (diagonal matmuls), DVE and GPSIMD.
"""

import sys

if "/opt/trn_rl_repo" not in sys.path:
    sys.path.insert(0, "/opt/trn_rl_repo")

import numpy as np

import concourse.bass as bass
import concourse.tile as tile
from concourse import bacc, mybir
from concourse import bass_utils

# ---------------------------------------------------------------- problem dims
B = 2
C = 256
H = 256
W = 256
HEADS = 8
N_CORES = 8
ROWS = H // N_CORES          # 32 owned rows per core
VB = ROWS + 2                # 34 v/out band rows (1-row halo each side)
XB = ROWS + 4                # 36 x/qkv band rows (2-row halo each side)
CT = C // 128                # 2 channel tiles of 128 per 256-ch tensor
QKCT = 4                     # q,k channel tiles (512 ch)
EPS = 1e-12

fp32 = mybir.dt.float32
fp32r = mybir.dt.float32r
bf16 = mybir.dt.bfloat16

# tap assignment (tap = dy*3+dx). dx=1 taps (1,4,7) are 2-byte offset in bf16
# -> no DVE 2x mode; route them to GP where possible.
QK_PE_TAPS = [0, 2, 5, 7]    # diagonal-matmul taps on the PE
QK_DVE_INIT = 3              # DVE op that also folds in the PE-tap psum
QK_GP_PAIRS = [1, 4]         # gpsimd mul+add pairs
QK_DVE = [6, 8]
V_PE_TAPS = [0, 1]
V_DVE = [2, 3, 4, 5, 6, 7, 8]

CHUNK = 8                    # QK-pass rows per chunk
VCHUNK = 8                   # V-pass rows per chunk (non-overlapping over VB)

Alu = mybir.AluOpType
Act = mybir.ActivationFunctionType

_CONST_POOL = None


def _single(tc, shape, dtype, name):
    return _CONST_POOL.tile(shape, dtype, tag=name, name=name)


def _chunks(total, step):
    out = []
    s = 0
    while s < total:
        out.append((s, min(step, total - s)))
        s += step
    return out


def build_program():
    nc = bacc.Bacc(
        "TRN2", target_bir_lowering=False, debug=False, num_devices=N_CORES
    )

    # ------------------------------------------------------------- DRAM I/O
    x_d = nc.dram_tensor("x", [B, CT, 128, XB, 256], fp32r, kind="ExternalInput")
    wq_d = nc.dram_tensor("wq", [128, CT, 768], fp32r, kind="ExternalInput")
    wdw_d = nc.dram_tensor("wdw", [128, 6, 9], fp32, kind="ExternalInput")
    wp_d = nc.dram_tensor("wp", [128, CT, 9, 256], fp32r, kind="ExternalInput")
    ident_d = nc.dram_tensor("ident", [128, 128], fp32r, kind="ExternalInput")
    resc_d = nc.dram_tensor("resc", [128, CT], fp32, kind="ExternalInput")
    hmask_d = nc.dram_tensor("hmask", [128, 2], fp32, kind="ExternalInput")

    vband_d = nc.dram_tensor(
        "vband", [B, CT, 128, VB, 256], fp32, kind="ExternalOutput"
    )
    outc_d = nc.dram_tensor(
        "outc", [B, CT, 128, ROWS, 256], fp32, kind="ExternalOutput"
    )

    with tile.TileContext(nc) as tc:
        global _CONST_POOL
        with tc.tile_pool(name="consts", bufs=1) as cpool:
            _CONST_POOL = cpool
            _build(nc, tc, x_d, wq_d, wdw_d, wp_d, ident_d, resc_d, hmask_d,
                   vband_d, outc_d)
            _CONST_POOL = None
    nc.compile()
    return nc


def _build(nc, tc, x_d, wq_d, wdw_d, wp_d, ident_d, resc_d, hmask_d,
           vband_d, outc_d):
    # ------------------------------------------------------ constants in SBUF
    wq = _single(tc, [128, CT, 768], fp32r, name="wq_sb")
    wdw = _single(tc, [128, 6, 9], fp32, name="wdw_sb")
    wp = _single(tc, [128, CT, 9, 256], fp32r, name="wp_sb")
    ident = _single(tc, [128, 128], fp32r, name="ident_sb")
    resc = _single(tc, [128, CT], fp32, name="resc_sb")
    hmask = _single(tc, [128, 2], fp32, name="hmask_sb")
    nc.sync.dma_start(wq[:], wq_d[:, :, :])
    nc.sync.dma_start(wdw[:], wdw_d[:, :, :])
    nc.sync.dma_start(wp[:], wp_d[:, :, :, :])
    nc.sync.dma_start(ident[:], ident_d[:, :])
    nc.sync.dma_start(resc[:], resc_d[:, :])
    nc.sync.dma_start(hmask[:], hmask_d[:, :])

    identb = _single(tc, [128, 128], bf16, name="identb_sb")
    nc.vector.tensor_copy(identb[:], ident[:].bitcast(fp32))

    # diagonal weight matrices for the PE depthwise taps
    diags = {}
    for t in range(CT):
        for tp in V_PE_TAPS:
            d = _single(tc, [128, 128], fp32r, name=f"diag_{t}_{tp}")
            nc.vector.tensor_scalar_mul(
                d[:], ident[:], wdw[:, QKCT + t, tp : tp + 1]
            )
            diags[(QKCT + t, tp)] = d
    for t in range(QKCT):
        for tp in QK_PE_TAPS:
            d = _single(tc, [128, 128], bf16, name=f"diagb_{t}_{tp}")
            nc.vector.tensor_scalar_mul(
                d[:], identb[:], wdw[:, t, tp : tp + 1]
            )
            diags[(t, tp)] = d

    # global accumulators
    stats = _single(tc, [128, 136], fp32, name="stats_sb")
    gacc = _single(tc, [128, B, 2, 256], fp32, name="gacc_sb")
    nc.gpsimd.memset(stats[:], 0.0)
    nc.gpsimd.memset(gacc[:], 0.0)

    # =========================================================== QK pass
    # owned v-band rows [1, 33): q,k (bf16), their sumsq, and the raw Gram.
    with (
        tc.tile_pool(name="xband", bufs=3) as p_x,
        tc.tile_pool(name="qkvt", bufs=9) as p_qkv,
        tc.tile_pool(name="qkp", bufs=9) as p_qk,
        tc.tile_pool(name="sqp", bufs=3) as p_sq,
        tc.tile_pool(name="scrp", bufs=2) as p_scr,
        tc.tile_pool(name="gsc", bufs=4) as p_gscr,
        tc.tile_pool(name="qtp", bufs=4) as p_qt,
        tc.tile_pool(name="psc", bufs=2, space="PSUM") as ps_conv,
        tc.tile_pool(name="psdq", bufs=2, space="PSUM") as ps_dwq,
        tc.tile_pool(name="pst", bufs=2, space="PSUM") as ps_tr,
        tc.tile_pool(name="psg", bufs=2, space="PSUM") as ps_gram,
    ):
        for b in range(B):
            for s, L in _chunks(ROWS, CHUNK):
                s += 1  # band rows [1, 33)
                x_qc = p_x.tile([128, CT, L + 2, 256], fp32r, tag="xq")
                for kt in range(CT):
                    nc.sync.dma_start(
                        x_qc[:, kt], x_d[b, kt][:, s : s + L + 2, :]
                    )
                qk_tiles = []
                for t in range(QKCT):
                    qkv_t = p_qkv.tile([128, L + 2, 258], bf16, tag="qkvt")
                    nc.gpsimd.memset(qkv_t[:, :, 0], 0.0)
                    nc.gpsimd.memset(qkv_t[:, :, 257], 0.0)
                    for n in range((L + 2) // 2):
                        ps = ps_conv.tile([128, 2, 256], fp32, tag="psc")
                        for kt in range(CT):
                            rhs = x_qc[:, kt, 2 * n : 2 * n + 2, :]
                            nc.tensor.matmul(
                                ps[:],
                                wq[:, kt, t * 128 : (t + 1) * 128],
                                rhs,
                                start=(kt == 0),
                                stop=(kt == CT - 1),
                            )
                        nc.scalar.copy(qkv_t[:, 2 * n : 2 * n + 2, 1:257], ps[:])
                    qk_t = p_qk.tile([128, L, 256], bf16, tag="qk")
                    qk3 = qk_t[:, :, :]

                    def win(tp, LL=L, src=qkv_t):
                        dy, dx = tp // 3, tp % 3
                        return src[:, dy : dy + LL, dx : dx + 256]

                    def sc(tp, tt=t):
                        return wdw[:, tt, tp : tp + 1]

                    # PE taps -> psum; DVE init op folds psum + tap 3;
                    # then gp pairs + remaining dve taps
                    dyi, dxi = QK_DVE_INIT // 3, QK_DVE_INIT % 3
                    for n in range((L * 256) // 512):
                        psd = ps_dwq.tile([128, 2, 256], fp32, tag="psdq")
                        for j, tp in enumerate(QK_PE_TAPS):
                            dy, dx = tp // 3, tp % 3
                            nc.tensor.matmul(
                                psd[:],
                                diags[(t, tp)][:],
                                qkv_t[:, 2 * n + dy : 2 * n + dy + 2,
                                      dx : dx + 256],
                                start=(j == 0),
                                stop=(j == len(QK_PE_TAPS) - 1),
                            )
                        nc.vector.scalar_tensor_tensor(
                            qk3[:, 2 * n : 2 * n + 2, :],
                            qkv_t[:, 2 * n + dyi : 2 * n + dyi + 2,
                                  dxi : dxi + 256],
                            sc(QK_DVE_INIT),
                            psd[:],
                            op0=Alu.mult, op1=Alu.add,
                        )
                    for tp in QK_GP_PAIRS:
                        gscr = p_gscr.tile([128, L, 256], bf16, tag="gscr")
                        nc.gpsimd.tensor_scalar_mul(gscr[:], win(tp), sc(tp))
                        nc.gpsimd.tensor_tensor(qk3, qk3, gscr[:], op=Alu.add)
                    for tp in QK_DVE:
                        nc.vector.scalar_tensor_tensor(
                            qk3, win(tp), sc(tp), qk3, op0=Alu.mult, op1=Alu.add
                        )
                    qk_tiles.append(qk_t)

                    # sumsq of this chunk -> stats col 128 + b*4 + t
                    scr = p_scr.tile([128, L, 256], bf16, tag="scr")
                    sq = p_sq.tile([128, 1], fp32, tag="sq")
                    nc.scalar.activation(
                        scr[:], qk_t[:], Act.Square, accum_out=sq[:]
                    )
                    col = 128 + b * 4 + t
                    nc.vector.tensor_tensor(
                        stats[:, col : col + 1],
                        stats[:, col : col + 1],
                        sq[:],
                        op=Alu.add,
                    )

                # transposes + Gram over this chunk's pixels
                nblk = (L * 256) // 128
                g_ps = [
                    ps_gram.tile([128, 256], fp32, tag="psg", name="gps")
                    for _ in range(2)
                ]
                for blk in range(nblk):
                    r, cb = blk // 2, (blk % 2) * 128
                    qt_t = p_qt.tile([128, 256], bf16, tag="qt")
                    kt_t = p_qt.tile([128, 256], bf16, tag="kt")
                    ps_q = ps_tr.tile([128, 256], bf16, tag="pst")
                    ps_k = ps_tr.tile([128, 256], bf16, tag="pst")
                    for half in range(2):
                        nc.tensor.matmul(
                            ps_q[:, half * 128 : half * 128 + 128],
                            qk_tiles[half][:, r, cb : cb + 128],
                            identb[:],
                            is_transpose=True,
                            skip_group_check=True,
                        )
                        nc.tensor.matmul(
                            ps_k[:, half * 128 : half * 128 + 128],
                            qk_tiles[2 + half][:, r, cb : cb + 128],
                            identb[:],
                            is_transpose=True,
                            skip_group_check=True,
                        )
                    nc.scalar.copy(qt_t[:], ps_q[:])
                    nc.scalar.copy(kt_t[:], ps_k[:])
                    for g in range(2):
                        nc.tensor.matmul(
                            g_ps[g][:],
                            kt_t[:, g * 128 : (g + 1) * 128],
                            qt_t[:],
                            start=(blk == 0),
                            stop=(blk == nblk - 1),
                            skip_group_check=True,
                        )
                for g in range(2):
                    nc.vector.tensor_tensor(
                        gacc[:, b, g, :], gacc[:, b, g, :], g_ps[g][:],
                        op=Alu.add,
                    )

    # extract per-head diagonal 32x32 blocks of the Gram into stats cols
    for b in range(B):
        for g in range(2):
            for i in range(4):
                h = 4 * g + i
                src = gacc[32 * i : 32 * i + 32, b, g, 32 * h : 32 * h + 32]
                dst = stats[32 * i : 32 * i + 32, (2 * b + g) * 32 :][:, :32]
                nc.vector.tensor_copy(dst, src)

    # ============================================================ AllReduce
    with tc.tile_pool(name="ardram", bufs=1, space="DRAM") as p_ar:
        ar_in = p_ar.tile([128, 136], fp32)
        ar_out = p_ar.tile([128, 136], fp32, addr_space="Shared")
        nc.sync.dma_start(ar_in[:], stats[:])
        nc.gpsimd.collective_compute(
            "AllReduce",
            Alu.add,
            replica_groups=[list(range(N_CORES))],
            ins=[ar_in[:].opt()],
            outs=[ar_out[:].opt()],
        )
        stats2 = _single(tc, [128, 136], fp32, name="stats2_sb")
        nc.sync.dma_start(stats2[:], ar_out[:])

    # ====================================================== softmax -> attnT
    # rsq[:, idx] = 1 / max(sqrt(sumsq), eps), idx = b*4 + qk*2 + g
    rsq = _single(tc, [128, 8], fp32, name="rsq_sb")
    nc.scalar.activation(rsq[:], stats2[:, 128:136], Act.Sqrt)
    nc.vector.tensor_scalar_max(rsq[:], rsq[:], EPS)
    nc.vector.reciprocal(rsq[:], rsq[:])

    bd = {}
    with tc.tile_pool(name="smx", bufs=4) as p_sm:
        for b in range(B):
            for g in range(2):
                kcol = b * 4 + 2 + g
                qcol = b * 4 + g
                ksc = p_sm.tile([128, 1], fp32, tag="ksc")
                nc.vector.tensor_tensor(
                    ksc[:], rsq[:, kcol : kcol + 1], resc[:, g : g + 1],
                    op=Alu.mult,
                )
                t1 = p_sm.tile([128, 32], fp32, tag="t1")
                graw = stats2[:, (2 * b + g) * 32 :][:, :32]
                nc.vector.tensor_scalar_mul(t1[:], graw, ksc[:])
                # M[p, j] = rsq_q[32*(p//32) + j]: broadcast + block-transpose
                a2 = p_sm.tile([128, 32], fp32, tag="a2")
                nc.vector.tensor_scalar(
                    a2[:], t1[:], 0.0, rsq[:, qcol : qcol + 1],
                    op0=Alu.mult, op1=Alu.add,
                )
                m = p_sm.tile([128, 32], fp32, tag="m")
                nc.vector.transpose(m[:], a2[:])
                nc.vector.tensor_tensor(t1[:], t1[:], m[:], op=Alu.mult)
                # softmax over the free (e) dim
                mx = p_sm.tile([128, 1], fp32, tag="mx")
                nc.vector.tensor_reduce(
                    mx[:], t1[:], mybir.AxisListType.X, Alu.max
                )
                nc.vector.tensor_scalar_sub(t1[:], t1[:], mx[:])
                ex = p_sm.tile([128, 32], fp32, tag="ex")
                nc.scalar.activation(ex[:], t1[:], Act.Exp)
                sm = p_sm.tile([128, 1], fp32, tag="sm")
                nc.vector.tensor_reduce(
                    sm[:], ex[:], mybir.AxisListType.X, Alu.add
                )
                nc.vector.reciprocal(sm[:], sm[:])
                at = p_sm.tile([128, 32], fp32, tag="at")
                nc.vector.tensor_scalar_mul(at[:], ex[:], sm[:])
                att = p_sm.tile([128, 32], fp32, tag="att")
                nc.vector.transpose(att[:], at[:])
                # block-diagonal lhsT for the attn@v matmul
                bdt = _single(tc, [128, 128], fp32r, name=f"bd_{b}_{g}")
                nc.gpsimd.memset(bdt[:].bitcast(fp32), 0.0)
                for i in range(4):
                    nc.vector.tensor_copy(
                        bdt[32 * i : 32 * i + 32, 32 * i : 32 * i + 32],
                        att[32 * i : 32 * i + 32, :],
                    )
                bd[(b, g)] = bdt

    # ================================================= V + attn + proj pass
    # Non-overlapping v chunks fill a persistent per-batch out band; then the
    # proj 3x3 sweeps the band.
    with (
        tc.tile_pool(name="xc2", bufs=2) as p_x2,
        tc.tile_pool(name="qkvt2", bufs=2) as p_qkv2,
        tc.tile_pool(name="vtp", bufs=3) as p_v,
        tc.tile_pool(name="outb", bufs=1) as p_out,
        tc.tile_pool(name="ocp", bufs=2) as p_oc,
        tc.tile_pool(name="psc2", bufs=2, space="PSUM") as ps_conv2,
        tc.tile_pool(name="psd2", bufs=2, space="PSUM") as ps_dw2,
        tc.tile_pool(name="psa", bufs=2, space="PSUM") as ps_attn,
        tc.tile_pool(name="psp", bufs=2, space="PSUM") as ps_proj,
    ):
        for b in range(B):
            o_band = p_out.tile([128, CT, VB, 258], fp32r, tag="oband")
            for g in range(CT):
                nc.gpsimd.memset(o_band[:, g, :, 0].bitcast(fp32), 0.0)
                nc.gpsimd.memset(o_band[:, g, :, 257].bitcast(fp32), 0.0)

            def proj_group(mt, grp):
                for half in range(2):
                    oc_t = p_oc.tile([128, 4, 256], fp32, tag="oc",
                                     name="oc_t")
                    for j in range(2):
                        n = grp * 4 + half * 2 + j
                        ps = ps_proj.tile([128, 2, 256], fp32, tag="psp",
                                          name="ps_pj")
                        idx = 0
                        for tp in range(9):
                            dy, dx = tp // 3, tp % 3
                            for kt in range(CT):
                                rhs = o_band[:, kt,
                                             2 * n + dy : 2 * n + dy + 2,
                                             dx : dx + 256]
                                nc.tensor.matmul(
                                    ps[:],
                                    wp[:, kt, tp, mt * 128 :][:, :128],
                                    rhs,
                                    start=(idx == 0),
                                    stop=(idx == 17),
                                    skip_group_check=True,
                                )
                                idx += 1
                        nc.scalar.copy(oc_t[:, 2 * j : 2 * j + 2, :], ps[:])
                    nc.sync.dma_start(
                        outc_d[b, mt][:, grp * 8 + half * 4 :][:, :4, :],
                        oc_t[:],
                    )

            for ci, (c0, Lv) in enumerate(_chunks(VB, VCHUNK)):
                LX = Lv + 2
                x_c = p_x2.tile([128, CT, LX, 256], fp32r, tag="xc")
                for kt in range(CT):
                    nc.sync.dma_start(
                        x_c[:, kt], x_d[b, kt][:, c0 : c0 + LX, :]
                    )

                v_tiles = []
                for t in range(CT):
                    qkv_t = p_qkv2.tile([128, LX, 258], fp32r, tag="qkvt2")
                    nc.gpsimd.memset(qkv_t[:, :, 0].bitcast(fp32), 0.0)
                    nc.gpsimd.memset(qkv_t[:, :, 257].bitcast(fp32), 0.0)
                    for n in range(LX // 2):
                        ps = ps_conv2.tile([128, 2, 256], fp32, tag="psc2")
                        for kt in range(CT):
                            rhs = x_c[:, kt, 2 * n : 2 * n + 2, :]
                            nc.tensor.matmul(
                                ps[:],
                                wq[:, kt, (QKCT + t) * 128 :][:, :128],
                                rhs,
                                start=(kt == 0),
                                stop=(kt == CT - 1),
                            )
                        nc.scalar.copy(qkv_t[:, 2 * n : 2 * n + 2, 1:257], ps[:])
                    v_t = p_v.tile([128, Lv, 256], fp32r, tag="vt")
                    v3 = v_t[:, :, :]
                    # PE taps -> psum -> evac, then DVE chain on top
                    for n in range((Lv * 256) // 512):
                        ps = ps_dw2.tile([128, 2, 256], fp32, tag="psdw")
                        for j, tp in enumerate(V_PE_TAPS):
                            dy, dx = tp // 3, tp % 3
                            rhs = qkv_t[:, 2 * n + dy : 2 * n + dy + 2,
                                        dx : dx + 256]
                            nc.tensor.matmul(
                                ps[:],
                                diags[(QKCT + t, tp)][:],
                                rhs,
                                start=(j == 0),
                                stop=(j == len(V_PE_TAPS) - 1),
                            )
                        nc.scalar.copy(v3[:, 2 * n : 2 * n + 2, :], ps[:])
                    for tp in V_DVE:
                        dy, dx = tp // 3, tp % 3
                        nc.vector.scalar_tensor_tensor(
                            v3, qkv_t[:, dy : dy + Lv, dx : dx + 256],
                            wdw[:, QKCT + t, tp : tp + 1], v3,
                            op0=Alu.mult, op1=Alu.add,
                        )
                    # halo masking at image edges
                    if c0 == 0:
                        nc.vector.tensor_scalar_mul(
                            v_t[:, 0, :], v_t[:, 0, :], hmask[:, 0:1]
                        )
                    if c0 + Lv == VB:
                        nc.vector.tensor_scalar_mul(
                            v_t[:, Lv - 1, :], v_t[:, Lv - 1, :], hmask[:, 1:2]
                        )
                    v_tiles.append(v_t)
                    # vband output: owned band rows [1, 33) only
                    lo = max(c0, 1)
                    hi = min(c0 + Lv, VB - 1)
                    if hi > lo:
                        nc.sync.dma_start(
                            vband_d[b, t][:, lo:hi, :],
                            v_t[:, lo - c0 : hi - c0, :].bitcast(fp32),
                        )

                # out band rows [c0, c0+Lv) = attn @ v
                for g in range(CT):
                    for n in range((Lv * 256) // 512):
                        ps = ps_attn.tile([128, 2, 256], fp32, tag="psa")
                        nc.tensor.matmul(
                            ps[:],
                            bd[(b, g)][:],
                            v_tiles[g][:, 2 * n : 2 * n + 2, :],
                            start=True,
                            stop=True,
                        )
                        nc.scalar.copy(
                            o_band[:, g, c0 + 2 * n : c0 + 2 * n + 2, 1:257],
                            ps[:],
                        )
                if ci >= 1:
                    for mt in range(CT):
                        proj_group(mt, ci - 1)


# ------------------------------------------------------------------- host side
_NC_CACHE = None


def _get_program():
    global _NC_CACHE
    if _NC_CACHE is None:
        _NC_CACHE = build_program()
    return _NC_CACHE


def kernel(x_in, w_qkv, w_dw, rescale, w_proj):
    x_in = np.asarray(x_in, dtype=np.float32)
    w_qkv = np.asarray(w_qkv, dtype=np.float32)
    w_dw = np.asarray(w_dw, dtype=np.float32)
    rescale = np.asarray(rescale, dtype=np.float32)
    w_proj = np.asarray(w_proj, dtype=np.float32)

    # x: NHWC -> NCHW, pad 2 halo rows top/bottom
    xT = np.transpose(x_in, (0, 3, 1, 2))                    # [B, C, H, W]
    xpad = np.zeros((B, C, H + 4, W), np.float32)
    xpad[:, :, 2 : H + 2, :] = xT

    # weights in device layouts
    wq_h = w_qkv[:, :, 0, 0]                                 # [768, 256]
    wq_l = np.ascontiguousarray(
        wq_h.T.reshape(CT, 128, 768).transpose(1, 0, 2)
    )                                                        # [128, CT, 768]
    wdw_l = np.ascontiguousarray(
        w_dw[:, 0].reshape(6, 128, 9).transpose(1, 0, 2)
    )                                                        # [128, 6, 9]
    wp_l = np.ascontiguousarray(
        w_proj.transpose(1, 2, 3, 0)                         # [i, 3, 3, o]
        .reshape(C, 9, C)
        .reshape(CT, 128, 9, C)
        .transpose(1, 0, 2, 3)
    )                                                        # [128, CT, 9, 256]
    ident = np.eye(128, dtype=np.float32)
    resc_l = np.empty((128, CT), np.float32)
    r = rescale.reshape(HEADS)
    for g in range(CT):
        resc_l[:, g] = np.repeat(r[4 * g : 4 * g + 4], 32)

    in_maps = []
    for i in range(N_CORES):
        band = np.ascontiguousarray(
            xpad[:, :, 32 * i : 32 * i + XB, :]
        ).reshape(B, CT, 128, XB, 256)
        hm = np.ones((128, 2), np.float32)
        if i == 0:
            hm[:, 0] = 0.0
        if i == N_CORES - 1:
            hm[:, 1] = 0.0
        in_maps.append(
            {
                "x": band,
                "wq": wq_l,
                "wdw": wdw_l,
                "wp": wp_l,
                "ident": ident,
                "resc": resc_l,
                "hmask": hm,
            }
        )

    nc = _get_program()
    res = bass_utils.run_bass_kernel_spmd(
        nc, in_maps, core_ids=list(range(N_CORES))
    )

    v_out = np.empty((B, C, H, W), np.float32)
    outc = np.empty((B, C, H, W), np.float32)
    for i in range(N_CORES):
        vb = res.results[i]["vband"]                 # [B, CT, 128, VB, 256]
        oc = res.results[i]["outc"]                  # [B, CT, 128, ROWS, 256]
        v_out[:, :, 32 * i : 32 * i + 32, :] = vb[:, :, :, 1:33, :].reshape(
            B, C, 32, 256
        )
        outc[:, :, 32 * i : 32 * i + 32, :] = oc.reshape(B, C, 32, 256)

    out_c = np.ascontiguousarray(np.transpose(outc, (0, 2, 3, 1)))
    return (out_c, v_out)



# revision 13
# speedup vs baseline: 1.3607x; 1.3607x over previous
"""Trainium2 Bass kernel for nn_Conv_MS_MSA (spectral multi-head self-attention).

Reference computation (per batch):
  qkv = dw3x3_depthwise(conv1x1(x))          # 256 -> 768 ch, then per-ch 3x3
  q, k, v = split(qkv); v_out = v
  per head (8 heads x 32 d): L2-normalize q,k rows over the 65536 pixels,
  attn = softmax(k_norm @ q_norm^T * rescale), out = attn @ v
  out_c = conv3x3_dense(out, w_proj)         # 256 -> 256 ch

Sharding: spatial bands; core i owns image rows [32i, 32i+32) of both batches.
Global coupling is only the per-head Gram matrices + q/k row norms, reduced
with one small AllReduce that overlaps the v-computation phase.

Key structure (vs a direct port):
  * attn is folded into the proj-conv weights (out_c = conv3x3(v, wp @ A)),
    so no attn@v pass and no separate "out" tensor exist.
  * the whole q/k path runs in fp8e4m3: the 1x1 conv and the depthwise taps
    use DoubleRow matmuls (2 contraction tiles / 2 taps per pass at 0.5
    cycles/row), and the Gram also runs as DoubleRow over pixel-block pairs.
    fp8 noise is zero-mean and averages out over the 65536-pixel contraction;
    per-row scale errors cancel exactly against the matching norms.
  * q/k sumsq (for the normalization) comes out of an extended Gram
    (k^T[q|k] and q^T q) instead of a separate activation pass.
  * the v path runs in bf16: 8 depthwise taps as PE diagonal matmuls, the
    9th tap folded into the PSUM->SBUF evacuation on the DVE.
"""

import sys

if "/opt/trn_rl_repo" not in sys.path:
    sys.path.insert(0, "/opt/trn_rl_repo")

import numpy as np

import bass_rust
import concourse.bass as bass
import concourse.tile as tile
from concourse import bacc, mybir
from concourse import bass_utils

# ---------------------------------------------------------------- problem dims
B = 2
C = 256
H = 256
W = 256
HEADS = 8
N_CORES = 8
ROWS = H // N_CORES          # 32 owned rows per core
VB = ROWS + 2                # 34 v band rows (1-row halo each side)
XB = ROWS + 4                # 36 x band rows (2-row halo each side)
CT = 2                       # channel tiles of 128 per 256-ch tensor
QT = 4                       # q,k channel tiles (512 ch)
L1 = 8                       # phase-1 chunk: owned rows per chunk
L2 = 8                       # phase-2 chunk: band rows per chunk
EPS = 1e-12

fp32 = mybir.dt.float32
fp32r = mybir.dt.float32r
bf16 = mybir.dt.bfloat16
fp8 = mybir.dt.float8e4

Alu = mybir.AluOpType
Act = mybir.ActivationFunctionType
DR = mybir.MatmulPerfMode.DoubleRow

# qk depthwise tap pairing: 4 DoubleRow pairs on PE + tap 8 folded on DVE
QK_PAIRS = [(0, 1), (2, 3), (4, 5), (6, 7)]
QK_FOLD = 8
# v depthwise: taps 1..8 on PE (bf16 diagonal matmuls), tap 0 folded on DVE
V_PE_TAPS = [1, 2, 3, 4, 5, 6, 7, 8]
V_FOLD = 0

_CONST_POOL = None


def _single(tc, shape, dtype, name):
    return _CONST_POOL.tile(shape, dtype, tag=name, name=name)


def _chunks(total, step):
    out = []
    s = 0
    while s < total:
        out.append((s, min(step, total - s)))
        s += step
    return out


def _pair_ap(t, tap_a, tap_b, r0, nrows, row_elems):
    """AP [128, 2, nrows, 256] over tile t's windows for taps a,b.

    Window for tap (dy, dx) at output row j is u[:, t, j + dy, dx : dx + 256]
    (u rows carry a 1-row halo so row index j+dy is in range). dim 1 walks
    from tap_a's window to tap_b's; any offset delta is legal for reads.
    """
    dya, dxa = tap_a // 3, tap_a % 3
    base = t[:, r0 + dya, dxa : dxa + 256]
    raw = [list(d) for d in base.ap]
    dyb, dxb = tap_b // 3, tap_b % 3
    delta = (dyb - dya) * row_elems + (dxb - dxa)
    return bass_rust.AP(base.tensor, base.offset,
                        [raw[0], [delta, 2]] + raw[1:])


def build_program():
    nc = bacc.Bacc(
        "TRN2", target_bir_lowering=False, debug=False, num_devices=N_CORES
    )

    # ------------------------------------------------------------- DRAM I/O
    x_d = nc.dram_tensor("x", [B, CT, 128, XB, 256], fp32r, kind="ExternalInput")
    x8_d = nc.dram_tensor("x8", [B, CT, 128, XB, 256], fp8, kind="ExternalInput")
    wqk_d = nc.dram_tensor("wqk", [128, CT, 512], fp8, kind="ExternalInput")
    wv_d = nc.dram_tensor("wv", [128, CT, 256], fp32r, kind="ExternalInput")
    wdw_d = nc.dram_tensor("wdw", [128, 6, 9], fp32, kind="ExternalInput")
    wp_d = nc.dram_tensor("wp", [128, CT, 9, 256], fp32r, kind="ExternalInput")
    dgq_d = nc.dram_tensor("dgq", [128, QT, 4, 2, 128], fp8, kind="ExternalInput")
    dgv_d = nc.dram_tensor("dgv", [128, CT, 8, 128], bf16, kind="ExternalInput")
    id8_d = nc.dram_tensor("id8", [128, 128], fp8, kind="ExternalInput")
    eyem_d = nc.dram_tensor("eyem", [128, 32], fp32, kind="ExternalInput")
    resc_d = nc.dram_tensor("resc", [128, CT], fp32, kind="ExternalInput")
    hmask_d = nc.dram_tensor("hmask", [128, 2], fp32, kind="ExternalInput")

    vout_d = nc.dram_tensor(
        "vout", [B, CT, 128, ROWS, 256], fp32, kind="ExternalOutput"
    )
    outc_d = nc.dram_tensor(
        "outc", [B, CT, 128, ROWS, 256], fp32, kind="ExternalOutput"
    )

    with tile.TileContext(nc) as tc:
        global _CONST_POOL
        with tc.tile_pool(name="consts", bufs=1) as cpool:
            _CONST_POOL = cpool
            _build(nc, tc, x_d, x8_d, wqk_d, wv_d, wdw_d, wp_d, dgq_d,
                   dgv_d, id8_d, eyem_d, resc_d, hmask_d, vout_d, outc_d)
            _CONST_POOL = None
    nc.compile()
    return nc


def _build(nc, tc, x_d, x8_d, wqk_d, wv_d, wdw_d, wp_d, dgq_d, dgv_d,
           id8_d, eyem_d, resc_d, hmask_d, vout_d, outc_d):
    # ------------------------------------------------------ constants in SBUF
    wqk8 = _single(tc, [128, CT, 512], fp8, name="wqk8")
    wv = _single(tc, [128, CT, 256], fp32r, name="wv_sb")
    wdw = _single(tc, [128, 6, 9], fp32, name="wdw_sb")
    dgq_sb = _single(tc, [128, QT, 4, 2, 128], fp8, name="dgq_sb")
    dgv_sb = _single(tc, [128, CT, 8, 128], bf16, name="dgv_sb")
    id8 = _single(tc, [128, 128], fp8, name="id8_sb")
    eyem = _single(tc, [128, 32], fp32, name="eyem_sb")
    resc = _single(tc, [128, CT], fp32, name="resc_sb")
    hmask = _single(tc, [128, 2], fp32, name="hmask_sb")
    nc.sync.dma_start(wqk8[:], wqk_d[:, :, :])
    nc.sync.dma_start(wv[:], wv_d[:, :, :])
    nc.sync.dma_start(wdw[:], wdw_d[:, :, :])
    nc.sync.dma_start(dgq_sb[:], dgq_d[:, :, :, :, :])
    nc.sync.dma_start(dgv_sb[:], dgv_d[:, :, :, :])
    nc.sync.dma_start(id8[:], id8_d[:, :])
    nc.sync.dma_start(eyem[:], eyem_d[:, :])
    nc.sync.dma_start(resc[:], resc_d[:, :])
    nc.sync.dma_start(hmask[:], hmask_d[:, :])

    dgq = {(t, pi): dgq_sb[:, t, pi]
           for t in range(QT) for pi in range(len(QK_PAIRS))}
    dgv = {(t, tp): dgv_sb[:, t, j]
           for t in range(CT) for j, tp in enumerate(V_PE_TAPS)}

    # gram results + AR stats
    gram_sb = _single(tc, [128, B, 2, 384], fp32, name="gram_sb")
    stats = _single(tc, [128, 136], fp32, name="stats_sb")

    # persistent bf16 v bands (proj input), per batch
    vband = _single(tc, [128, B, CT, VB, 258], bf16, name="vband_sb")
    for b in range(B):
        for t in range(CT):
            nc.gpsimd.memset(vband[:, b, t, :, 0], 0.0)
            nc.gpsimd.memset(vband[:, b, t, :, 257], 0.0)

    # =========================================================== Phase 1: q/k
    with (
        tc.tile_pool(name="x8p", bufs=2) as p_x8,
        tc.tile_pool(name="u8p", bufs=2) as p_u8,
        tc.tile_pool(name="qk8p", bufs=2) as p_qk,
        tc.tile_pool(name="qktp", bufs=2) as p_qkt,
        tc.tile_pool(name="psu1", bufs=2, space="PSUM") as ps_u1,
        tc.tile_pool(name="psd1", bufs=2, space="PSUM") as ps_d1,
        tc.tile_pool(name="pst1", bufs=2, space="PSUM") as ps_t1,
        tc.tile_pool(name="psg1", bufs=2, space="PSUM") as ps_g1,
    ):
        for b in range(B):
            g_ps = [ps_g1.tile([128, 384], fp32, tag="gps", name="gps")
                    for _ in range(2)]
            nchunks = ROWS // L1
            for ci in range(nchunks):
                c = ci * L1
                # x rows: u rows are owned [c-1, c+L1+1) -> x idx [c+1, c+11)
                x8 = p_x8.tile([128, CT, L1 + 2, 256], fp8, tag="x8")
                for kt in range(CT):
                    nc.sync.dma_start(
                        x8[:, kt], x8_d[b, kt][:, c + 1 : c + L1 + 3, :])

                # 1x1 conv -> u8 (rows [c-1, c+L1+1), 1-col zero halo)
                u8 = p_u8.tile([128, QT, L1 + 2, 258], fp8, tag="u8")
                nc.gpsimd.memset(u8[:, :, :, 0], 0.0)
                nc.gpsimd.memset(u8[:, :, :, 257], 0.0)
                for t in range(QT):
                    for n in range((L1 + 2) // 2):
                        psu = ps_u1.tile([128, 2, 256], fp32, tag="psu")
                        nc.tensor.matmul(
                            psu[:],
                            wqk8[:, :, t * 128 : (t + 1) * 128],
                            x8[:, :, 2 * n : 2 * n + 2, :]
                            .rearrange("p c r w -> p c (r w)"),
                            start=True, stop=True, perf_mode=DR,
                        )
                        if (n + t) % 2:
                            nc.scalar.copy(
                                u8[:, t, 2 * n : 2 * n + 2, 1:257], psu[:])
                        else:
                            nc.gpsimd.tensor_copy(
                                u8[:, t, 2 * n : 2 * n + 2, 1:257], psu[:])

                # depthwise 3x3 -> qk8 (owned rows [c, c+L1))
                qk8 = p_qk.tile([128, QT, L1, 256], fp8, tag="qk8")
                u_row = 258
                for t in range(QT):
                    for n in range(L1 // 2):
                        psd = ps_d1.tile([128, 2, 256], fp32, tag="psd")
                        for rr in range(2):
                            r = 2 * n + rr
                            for pi, (ta, tb) in enumerate(QK_PAIRS):
                                nc.tensor.matmul(
                                    psd[:, rr],
                                    dgq[(t, pi)],
                                    _pair_ap(u8[:, t], ta, tb, r, 1, u_row),
                                    start=(pi == 0),
                                    stop=(pi == len(QK_PAIRS) - 1),
                                    perf_mode=DR, skip_group_check=True,
                                )
                        dy, dx = QK_FOLD // 3, QK_FOLD % 3
                        nc.vector.scalar_tensor_tensor(
                            qk8[:, t, 2 * n : 2 * n + 2, :],
                            u8[:, t, 2 * n + dy : 2 * n + dy + 2, dx : dx + 256],
                            wdw[:, t, QK_FOLD : QK_FOLD + 1],
                            psd[:],
                            op0=Alu.mult, op1=Alu.add,
                        )

                # transposes: [128ch, 128pix] blocks -> qkt[g][:, blk, q|k]
                nblk = (L1 * 256) // 128
                qkt = [p_qkt.tile([128, nblk, 256], fp8, tag=f"qkt{g}",
                                  name=f"qkt{g}")
                       for g in range(2)]
                for g in range(2):
                    for blk in range(nblk):
                        r, cb = blk // 2, (blk % 2) * 128
                        pst = ps_t1.tile([128, 256], fp8, tag="pst")
                        nc.tensor.matmul(
                            pst[:, 0:128], qk8[:, g, r, cb : cb + 128],
                            id8[:], is_transpose=True, skip_group_check=True,
                        )
                        nc.tensor.matmul(
                            pst[:, 128:256], qk8[:, 2 + g, r, cb : cb + 128],
                            id8[:], is_transpose=True, skip_group_check=True,
                        )
                        nc.gpsimd.tensor_copy(qkt[g][:, blk, :], pst[:])

                # gram (DoubleRow over pixel-block pairs), accumulated over
                # the whole batch:  k^T [q|k]  and  q^T q
                for g in range(2):
                    for p in range(nblk // 2):
                        first = ci == 0 and p == 0
                        last = ci == nchunks - 1 and p == nblk // 2 - 1
                        nc.tensor.matmul(
                            g_ps[g][:, 0:256],
                            qkt[g][:, 2 * p : 2 * p + 2, 128:256],
                            qkt[g][:, 2 * p : 2 * p + 2, :],
                            start=first, stop=last,
                            perf_mode=DR, skip_group_check=True,
                        )
                        nc.tensor.matmul(
                            g_ps[g][:, 256:384],
                            qkt[g][:, 2 * p : 2 * p + 2, 0:128],
                            qkt[g][:, 2 * p : 2 * p + 2, 0:128],
                            start=first, stop=last,
                            perf_mode=DR, skip_group_check=True,
                        )
            for g in range(2):
                nc.scalar.copy(gram_sb[:, b, g, :], g_ps[g][:])

    # ------------------------------------------------- stats for the AllReduce
    # layout: cols 0:128   = per-(b,g) kq diag 32-blocks (4 col-groups of 32)
    #         cols 128:132 = sumsq_k (b*2+g), cols 132:136 = sumsq_q
    with tc.tile_pool(name="stx", bufs=2) as p_st:
        for b in range(B):
            for g in range(2):
                kq = gram_sb[:, b, g, :]
                for i in range(4):
                    nc.vector.tensor_copy(
                        stats[32 * i : 32 * i + 32, (b * 2 + g) * 32 :][:, :32],
                        kq[32 * i : 32 * i + 32, 32 * i : 32 * i + 32],
                    )
                kk = p_st.tile([128, 32], fp32, tag="kk")
                qq = p_st.tile([128, 32], fp32, tag="qq")
                for i in range(4):
                    nc.vector.tensor_copy(
                        kk[32 * i : 32 * i + 32, :],
                        kq[32 * i : 32 * i + 32, 128 + 32 * i :][:, :32],
                    )
                    nc.vector.tensor_copy(
                        qq[32 * i : 32 * i + 32, :],
                        kq[32 * i : 32 * i + 32, 256 + 32 * i :][:, :32],
                    )
                nc.vector.tensor_tensor(kk[:], kk[:], eyem[:], op=Alu.mult)
                nc.vector.tensor_tensor(qq[:], qq[:], eyem[:], op=Alu.mult)
                nc.vector.tensor_reduce(
                    stats[:, 128 + b * 2 + g : 129 + b * 2 + g], kk[:],
                    mybir.AxisListType.X, Alu.add)
                nc.vector.tensor_reduce(
                    stats[:, 132 + b * 2 + g : 133 + b * 2 + g], qq[:],
                    mybir.AxisListType.X, Alu.add)

    # ============================================================ AllReduce
    with tc.tile_pool(name="ardram", bufs=1, space="DRAM") as p_ar:
        ar_in = p_ar.tile([128, 136], fp32)
        ar_out = p_ar.tile([128, 136], fp32, addr_space="Shared")
        nc.sync.dma_start(ar_in[:], stats[:])
        nc.gpsimd.collective_compute(
            "AllReduce",
            Alu.add,
            replica_groups=[list(range(N_CORES))],
            ins=[ar_in[:].opt()],
            outs=[ar_out[:].opt()],
        )
        stats2 = _single(tc, [128, 136], fp32, name="stats2_sb")
        nc.sync.dma_start(stats2[:], ar_out[:])

    # ========================================================== Phase 2: v
    with (
        tc.tile_pool(name="xc2", bufs=2) as p_x2,
        tc.tile_pool(name="u16p", bufs=2) as p_u16,
        tc.tile_pool(name="v32p", bufs=2) as p_v32,
        tc.tile_pool(name="psu2", bufs=2, space="PSUM") as ps_u2,
        tc.tile_pool(name="psv2", bufs=2, space="PSUM") as ps_v2,
    ):
        for b in range(B):
            for c0, Lv in _chunks(VB, L2):
                LU = Lv + 2
                # u rows band-rel [c0-1, c0+Lv+1) -> x idx [c0, c0+Lv+2)
                x2 = p_x2.tile([128, CT, LU, 256], fp32r, tag="x2")
                for kt in range(CT):
                    nc.sync.dma_start(
                        x2[:, kt], x_d[b, kt][:, c0 : c0 + LU, :])
                u16 = p_u16.tile([128, CT, LU, 258], bf16, tag="u16")
                nc.vector.memset(u16[:, :, :, 0], 0.0)
                nc.vector.memset(u16[:, :, :, 257], 0.0)
                for t in range(CT):
                    for n in range(LU // 2):
                        psu = ps_u2.tile([128, 2, 256], fp32, tag="psu2")
                        for kt in range(CT):
                            nc.tensor.matmul(
                                psu[:],
                                wv[:, kt, t * 128 : (t + 1) * 128],
                                x2[:, kt, 2 * n : 2 * n + 2, :],
                                start=(kt == 0), stop=(kt == CT - 1),
                            )
                        nc.scalar.copy(u16[:, t, 2 * n : 2 * n + 2, 1:257],
                                       psu[:])

                v32 = p_v32.tile([128, CT, Lv, 256], fp32, tag="v32")
                for t in range(CT):
                    for n in range((Lv + 1) // 2):
                        nr = min(2, Lv - 2 * n)
                        psv = ps_v2.tile([128, 2, 256], fp32, tag="psv")
                        for j, tp in enumerate(V_PE_TAPS):
                            dy, dx = tp // 3, tp % 3
                            nc.tensor.matmul(
                                psv[:, :nr],
                                dgv[(t, tp)][:],
                                u16[:, t, 2 * n + dy : 2 * n + dy + nr,
                                    dx : dx + 256],
                                start=(j == 0), stop=(j == len(V_PE_TAPS) - 1),
                            )
                        dy, dx = V_FOLD // 3, V_FOLD % 3
                        nc.vector.scalar_tensor_tensor(
                            v32[:, t, 2 * n : 2 * n + nr, :],
                            u16[:, t, 2 * n + dy : 2 * n + dy + nr,
                                dx : dx + 256],
                            wdw[:, QT + t, V_FOLD : V_FOLD + 1],
                            psv[:, :nr],
                            op0=Alu.mult, op1=Alu.add,
                        )
                    # image-edge halo rows of the band must be zero for proj
                    if c0 == 0:
                        nc.vector.tensor_scalar_mul(
                            v32[:, t, 0, :], v32[:, t, 0, :], hmask[:, 0:1])
                    if c0 + Lv == VB:
                        nc.vector.tensor_scalar_mul(
                            v32[:, t, Lv - 1, :], v32[:, t, Lv - 1, :],
                            hmask[:, 1:2])
                    # owned rows -> v_out DMA (band rows [1, 33))
                    lo, hi = max(c0, 1), min(c0 + Lv, VB - 1)
                    if hi > lo:
                        nc.sync.dma_start(
                            vout_d[b, t][:, lo - 1 : hi - 1, :],
                            v32[:, t, lo - c0 : hi - c0, :],
                        )
                    nc.vector.tensor_copy(
                        vband[:, b, t, c0 : c0 + Lv, 1:257], v32[:, t])

    # ================================================== softmax + proj weights
    rsq = _single(tc, [128, 8], fp32, name="rsq_sb")
    nc.scalar.activation(rsq[:], stats2[:, 128:136], Act.Sqrt)
    nc.vector.tensor_scalar_max(rsq[:], rsq[:], EPS)
    nc.vector.reciprocal(rsq[:], rsq[:])

    with (
        tc.tile_pool(name="late", bufs=1) as p_late,
        tc.tile_pool(name="smx", bufs=4) as p_sm,
        tc.tile_pool(name="psw", bufs=2, space="PSUM") as ps_w,
        tc.tile_pool(name="oc4", bufs=2) as p_oc,
        tc.tile_pool(name="pso4", bufs=4, space="PSUM") as ps_o,
    ):
        wp = p_late.tile([128, CT, 9, 256], fp32r, tag="wp", name="wp_sb")
        nc.sync.dma_start(wp[:], wp_d[:, :, :, :])
        wpp = p_late.tile([128, B, CT, 9, 256], bf16, tag="wpp", name="wpp_sb")
        for b in range(B):
            for g in range(2):
                kcol, qcol = b * 2 + g, 4 + b * 2 + g
                ksc = p_sm.tile([128, 1], fp32, tag="ksc")
                nc.vector.tensor_tensor(
                    ksc[:], rsq[:, kcol : kcol + 1], resc[:, g : g + 1],
                    op=Alu.mult)
                t1 = p_sm.tile([128, 32], fp32, tag="t1")
                graw = stats2[:, (b * 2 + g) * 32 :][:, :32]
                nc.vector.tensor_scalar_mul(t1[:], graw, ksc[:])
                a2 = p_sm.tile([128, 32], fp32, tag="a2")
                nc.vector.tensor_scalar(
                    a2[:], t1[:], 0.0, rsq[:, qcol : qcol + 1],
                    op0=Alu.mult, op1=Alu.add)
                m = p_sm.tile([128, 32], fp32, tag="m")
                nc.vector.transpose(m[:], a2[:])
                nc.vector.tensor_tensor(t1[:], t1[:], m[:], op=Alu.mult)
                mx = p_sm.tile([128, 1], fp32, tag="mx")
                nc.vector.tensor_reduce(mx[:], t1[:], mybir.AxisListType.X,
                                        Alu.max)
                nc.vector.tensor_scalar_sub(t1[:], t1[:], mx[:])
                ex = p_sm.tile([128, 32], fp32, tag="ex")
                nc.scalar.activation(ex[:], t1[:], Act.Exp)
                sm = p_sm.tile([128, 1], fp32, tag="sm")
                nc.vector.tensor_reduce(sm[:], ex[:], mybir.AxisListType.X,
                                        Alu.add)
                nc.vector.reciprocal(sm[:], sm[:])
                at = p_sm.tile([128, 32], fp32, tag="at")
                nc.vector.tensor_scalar_mul(at[:], ex[:], sm[:])
                # block-diagonal A (lhsT: contract dim d on partitions)
                bdw = p_sm.tile([128, 128], fp32r, tag="bdw", name="bdw")
                nc.gpsimd.memset(bdw[:].bitcast(fp32), 0.0)
                for i in range(4):
                    nc.vector.tensor_copy(
                        bdw[32 * i : 32 * i + 32, 32 * i : 32 * i + 32],
                        at[32 * i : 32 * i + 32, :])
                # W'[e, tap, o] = sum_d A[d, e] wp[d, tap, o]
                for q in range(5):
                    f0, f1 = q * 512, min((q + 1) * 512, 2304)
                    psw = ps_w.tile([128, 512], fp32, tag="psw")
                    nc.tensor.matmul(
                        psw[:, : f1 - f0],
                        bdw[:],
                        wp[:, g].rearrange("p a b -> p (a b)")[:, f0:f1],
                        start=True, stop=True,
                    )
                    nc.scalar.copy(
                        wpp[:, b, g].rearrange("p a b -> p (a b)")[:, f0:f1],
                        psw[:, : f1 - f0])

        # ======================================================== Phase 4: proj
        for b in range(B):
            for mt in range(CT):
                for grp in range(4):           # 8 owned rows per group
                    oc = p_oc.tile([128, 8, 256], fp32, tag="oc")
                    for j in range(4):
                        n = grp * 4 + j        # owned row-pair index
                        pso = ps_o.tile([128, 2, 256], fp32, tag="pso")
                        idx = 0
                        for tp in range(9):
                            dy, dx = tp // 3, tp % 3
                            for g in range(CT):
                                # owned rows (2n, 2n+1) = band rows +1
                                nc.tensor.matmul(
                                    pso[:],
                                    wpp[:, b, g, tp, mt * 128 :][:, :128],
                                    vband[:, b, g, 2 * n + dy : 2 * n + dy + 2,
                                          dx : dx + 256],
                                    start=(idx == 0), stop=(idx == 17),
                                    skip_group_check=True,
                                )
                                idx += 1
                        nc.scalar.copy(oc[:, 2 * j : 2 * j + 2, :], pso[:])
                    nc.sync.dma_start(
                        outc_d[b, mt][:, grp * 8 : grp * 8 + 8, :], oc[:])


# ------------------------------------------------------------------- host side
_NC_CACHE = None


def _get_program():
    global _NC_CACHE
    if _NC_CACHE is None:
        _NC_CACHE = build_program()
    return _NC_CACHE


def kernel(x_in, w_qkv, w_dw, rescale, w_proj):
    import ml_dtypes
    f8 = ml_dtypes.float8_e4m3
    f16 = ml_dtypes.bfloat16

    x_in = np.asarray(x_in, dtype=np.float32)
    w_qkv = np.asarray(w_qkv, dtype=np.float32)
    w_dw = np.asarray(w_dw, dtype=np.float32)
    rescale = np.asarray(rescale, dtype=np.float32)
    w_proj = np.asarray(w_proj, dtype=np.float32)

    # x: NHWC -> NCHW, pad 2 halo rows top/bottom
    xT = np.transpose(x_in, (0, 3, 1, 2))                    # [B, C, H, W]
    xpad = np.zeros((B, C, H + 4, W), np.float32)
    xpad[:, :, 2 : H + 2, :] = xT

    # weights in device layouts (lhsT: in-ch on partitions)
    wq_h = w_qkv[:, :, 0, 0]                                 # [768, 256]
    wqk_l = np.ascontiguousarray(
        wq_h[:512].T.reshape(CT, 128, 512).transpose(1, 0, 2)).astype(f8)
    wv_l = np.ascontiguousarray(
        wq_h[512:].T.reshape(CT, 128, 256).transpose(1, 0, 2))
    wdw_l = np.ascontiguousarray(
        w_dw[:, 0].reshape(6, 128, 9).transpose(1, 0, 2))   # [128, 6, 9]
    wp_l = np.ascontiguousarray(
        w_proj.transpose(1, 2, 3, 0)                         # [i, 3, 3, o]
        .reshape(C, 9, C)
        .reshape(CT, 128, 9, C)
        .transpose(1, 0, 2, 3))                              # [128, CT, 9, 256]
    # diagonal depthwise tap matrices
    eye = np.eye(128, dtype=np.float32)
    dgq_l = np.zeros((128, QT, len(QK_PAIRS), 2, 128), np.float32)
    for t in range(QT):
        for pi, (ta, tb) in enumerate(QK_PAIRS):
            dgq_l[:, t, pi, 0] = eye * w_dw[128 * t : 128 * t + 128, 0,
                                            ta // 3, ta % 3][:, None]
            dgq_l[:, t, pi, 1] = eye * w_dw[128 * t : 128 * t + 128, 0,
                                            tb // 3, tb % 3][:, None]
    dgv_l = np.zeros((128, CT, len(V_PE_TAPS), 128), np.float32)
    for t in range(CT):
        for j, tp in enumerate(V_PE_TAPS):
            dgv_l[:, t, j] = eye * w_dw[512 + 128 * t : 640 + 128 * t, 0,
                                        tp // 3, tp % 3][:, None]
    eyem = np.zeros((128, 32), np.float32)
    for p in range(128):
        eyem[p, p % 32] = 1.0
    resc_l = np.empty((128, CT), np.float32)
    r = rescale.reshape(HEADS)
    for g in range(CT):
        resc_l[:, g] = np.repeat(r[4 * g : 4 * g + 4], 32)

    in_maps = []
    for i in range(N_CORES):
        band = np.ascontiguousarray(
            xpad[:, :, 32 * i : 32 * i + XB, :]
        ).reshape(B, CT, 128, XB, 256)
        hm = np.ones((128, 2), np.float32)
        if i == 0:
            hm[:, 0] = 0.0
        if i == N_CORES - 1:
            hm[:, 1] = 0.0
        in_maps.append(
            {
                "x": band,
                "x8": band.astype(f8),
                "wqk": wqk_l,
                "wv": wv_l,
                "wdw": wdw_l,
                "wp": wp_l,
                "dgq": dgq_l.astype(f8),
                "dgv": dgv_l.astype(f16),
                "id8": eye.astype(f8),
                "eyem": eyem,
                "resc": resc_l,
                "hmask": hm,
            }
        )

    nc = _get_program()
    res = bass_utils.run_bass_kernel_spmd(
        nc, in_maps, core_ids=list(range(N_CORES))
    )

    v_out = np.empty((B, C, H, W), np.float32)
    outc = np.empty((B, C, H, W), np.float32)
    for i in range(N_CORES):
        vb = res.results[i]["vout"]                  # [B, CT, 128, ROWS, 256]
        oc = res.results[i]["outc"]
        v_out[:, :, 32 * i : 32 * i + 32, :] = vb.reshape(B, C, 32, 256)
        outc[:, :, 32 * i : 32 * i + 32, :] = oc.reshape(B, C, 32, 256)

    out_c = np.ascontiguousarray(np.transpose(outc, (0, 2, 3, 1)))
    return (out_c, v_out)
